# revision 65
# baseline (speedup 1.0000x reference)
"""Trainium2 Bass kernel for the GTS spike-decoding GRU-DCRNN cell.

Strategy (8 NeuronCores, SPMD):
 - Destination-node sharding: 2500 real + 60 pad dest slots per core,
   bin-packed into 40 blocks x 64 dests so each block has <= 1024 in-edges.
 - CNN encoder runs feature-major per core; BN stats via tiny AllReduce.
 - Graph propagation: indirect-DMA row gathers (128 edges/instr) from a
   replicated node-major source matrix in DRAM, reduced by PE matmuls
   against selector matrices S (out-dir norm_out and in-dir norm_in
   folded into S values), PSUM-accumulated per dest block.
 - S is built ON DEVICE from compact per-edge (slot, norm_out, norm_in)
   vectors (49x less host->device traffic than shipping S dense).
 - Gathers use gpsimd.dma_gather (one SWDGE instruction per 512-1024 rows
   instead of one indirect DMA per 128 rows; int16 indices wrapped in 16
   partitions and replicated across the 8 Q7 partition groups).
 - Node-major hop outputs are AllGathered between hops (NCH row-chunks per
   collective; NCH=1 measured fastest -- this fabric has a large fixed
   cost per collective); feature-major transposes are spilled to DRAM and
   streamed as dense-gate matmul rhs with slab-major loop order so each
   slab is read once per column strip.
 - _build_nc(loop=K) emits the body K times (iterations serialized via
   cross-iteration deps) for amortized HW timing in test.py.
"""

import numpy as np
import ml_dtypes

import concourse.bass as bass
import concourse.tile as tile
from concourse import bass_utils, mybir, bacc
from bass_rust import add_dep_helper

N_NODES = 20000
N_EDGES = 320000
EMB = 256
BN_EPS = 1e-5
N_CORES = 8
NPC = N_NODES // N_CORES
NPAD = 2560
NBLK = 40
BLK = 64
CPB = 8
NCHUNK = NBLK * CPB
L_IN = 100
L1 = 31
L2 = 8
C1 = 32
XPAD = 112
F1 = C1 * L1
NTOT = N_CORES * NPAD
NCH = 1                      # collective pipeline chunks per AllGather
CROWS = NPAD // NCH          # 640 rows per chunk per core
GROWS = N_CORES * CROWS      # 5120 rows per chunk in the gathered layout

bf16 = mybir.dt.bfloat16
f32 = mybir.dt.float32
i32 = mybir.dt.int32
AF = mybir.ActivationFunctionType
OP = mybir.AluOpType


def _split_multi_waits(nc):
    """This walrus rejects instructions with >1 semaphore wait. Split extra
    waits onto single-wait NoOps inserted just before, same engine."""
    ctr = 0
    for f in nc.m.functions:
        for bb in f.blocks:
            insts = bb.instructions
            if not any(i.sync_info is not None and len(i.sync_info.on_wait) > 1
                       for i in insts):
                continue
            new_list = []
            for inst in insts:
                si = inst.sync_info
                waits = list(si.on_wait) if si is not None else []
                if len(waits) > 1:
                    for w in waits[:-1]:
                        ctr += 1
                        nop = mybir.InstNoOp(name=f"splitw-{ctr}",
                                             text_hint="splitw")
                        nop.engine = inst.engine
                        nop.sync_info = mybir.SyncInfo(on_wait=[w], on_update=[])
                        new_list.append(nop)
                    si.on_wait = waits[-1:]
                new_list.append(inst)
            bb.instructions = new_list
    return ctr


# =========================== host preprocessing ===========================

def _host_prep(x, edge_index, hidden_state, conv1_w, conv1_b, bn1_gamma,
               bn1_beta, conv2_w, conv2_b, bn2_gamma, bn2_beta,
               W_z, b_z, W_r, b_r, W_h, b_h):
    row = np.asarray(edge_index[0], np.int64)
    col = np.asarray(edge_index[1], np.int64)
    deg_out = np.bincount(row, minlength=N_NODES).astype(np.float64)
    deg_in = np.bincount(col, minlength=N_NODES).astype(np.float64)
    norm_out = (1.0 / np.maximum(deg_out, 1))[row].astype(np.float32)
    norm_in = (1.0 / np.maximum(deg_in, 1))[col].astype(np.float32)

    slot_of = np.full(N_NODES, -1, np.int64)
    node_of = np.full((N_CORES, NPAD), -1, np.int64)
    for c in range(N_CORES):
        nodes = np.arange(c * NPC, (c + 1) * NPC)
        degs = deg_in[nodes].astype(np.int64)
        order = np.argsort(-degs, kind="stable")
        bin_load = np.zeros(NBLK, np.int64)
        bin_cnt = np.zeros(NBLK, np.int64)
        for idx in order:
            d = int(degs[idx])
            cand = np.nonzero(bin_cnt < BLK)[0]
            ok = cand[(bin_load[cand] + d) <= CPB * 128]
            if len(ok) == 0:
                raise RuntimeError("bin packing overflow; raise CPB")
            b = ok[np.argmin(bin_load[ok])]
            slot_of[nodes[idx]] = b * BLK + bin_cnt[b]
            node_of[c, b * BLK + bin_cnt[b]] = nodes[idx]
            bin_load[b] += d
            bin_cnt[b] += 1
    # gathered matrices use a chunk-major layout: [NCH][core][CROWS] so each
    # AllGather chunk is a contiguous slab produced/consumed independently
    globalpos = np.full(N_NODES, -1, np.int64)
    for c in range(N_CORES):
        m = node_of[c] >= 0
        slots = np.nonzero(m)[0]
        globalpos[node_of[c][m]] = ((slots // CROWS) * GROWS + c * CROWS
                                    + (slots % CROWS))

    core_of_edge = col // NPC
    gidx_all = np.zeros((N_CORES, 16, NBLK * CPB * 8), np.int16)
    loc_all = np.zeros((N_CORES, 128, NCHUNK), ml_dtypes.bfloat16)
    no_all = np.zeros((N_CORES, 128, NCHUNK), ml_dtypes.bfloat16)
    ni_all = np.zeros((N_CORES, 128, NCHUNK), ml_dtypes.bfloat16)
    for c in range(N_CORES):
        eids = np.nonzero(core_of_edge == c)[0]
        dslot = slot_of[col[eids]]
        blk = dslot // BLK
        order = np.argsort(blk, kind="stable")
        eids = eids[order]
        blk = blk[order]
        loc_c = np.zeros((NCHUNK, 128), np.float32)
        no_c = np.zeros((NCHUNK, 128), np.float32)
        ni_c = np.zeros((NCHUNK, 128), np.float32)
        rows_c = np.zeros((NCHUNK, 128), np.int32)
        starts = np.searchsorted(blk, np.arange(NBLK + 1))
        for b in range(NBLK):
            be = eids[starts[b]:starts[b + 1]]
            ne = len(be)
            loc = slot_of[col[be]] - b * BLK
            gsrc = globalpos[row[be]]
            no = norm_out[be]
            ni = norm_in[be]
            for j in range(CPB):
                lo = j * 128
                if lo >= ne:
                    break
                hi = min(lo + 128, ne)
                k = hi - lo
                ch = b * CPB + j
                rows_c[ch, :k] = gsrc[lo:hi]
                loc_c[ch, :k] = loc[lo:hi]
                no_c[ch, :k] = no[lo:hi]
                ni_c[ch, :k] = ni[lo:hi]
        # flat per-block index list: position i in block b is chunk i//128,
        # partition i%128; dma_gather wraps flat index i at [i%16, i//16].
        flat = rows_c.reshape(NBLK, CPB * 128).astype(np.int16)
        gidx_all[c] = flat.reshape(NBLK, CPB * 8, 16).transpose(2, 0, 1)\
            .reshape(16, NBLK * CPB * 8)
        loc_all[c] = loc_c.T.astype(ml_dtypes.bfloat16)
        no_all[c] = no_c.T.astype(ml_dtypes.bfloat16)
        ni_all[c] = ni_c.T.astype(ml_dtypes.bfloat16)

    x2 = np.asarray(x, np.float32).reshape(N_NODES, L_IN)
    h0 = np.asarray(hidden_state, np.float32)
    xT_sl = np.zeros((N_CORES, XPAD, NPAD), ml_dtypes.bfloat16)
    h_sl = np.zeros((N_CORES, NPAD, EMB), ml_dtypes.bfloat16)
    maskv = np.zeros((N_CORES, NPAD), ml_dtypes.bfloat16)
    for c in range(N_CORES):
        m = node_of[c] >= 0
        xT_sl[c, :L_IN, m] = x2[node_of[c][m]]
        h_sl[c, m] = h0[node_of[c][m]]
        maskv[c, m] = 1.0

    w1 = np.asarray(conv1_w, np.float32)
    w2 = np.asarray(conv2_w, np.float32)
    W1t = np.zeros((XPAD, F1), np.float32)
    for l in range(L1):
        W1t[3 * l:3 * l + 10, l::L1] = w1[:, 0, :].T
    b1vec = np.repeat(np.asarray(conv1_b, np.float32), L1)[:, None]
    W2t = np.zeros((F1, C1 * L2), np.float32)
    for lo in range(L2):
        for k in range(10):
            li = 3 * lo + k
            W2t[li::L1, lo::L2] = w2[:, :, k].T
    WsumT = w2.sum(axis=2).T.copy()
    b2c = np.asarray(conv2_b, np.float32)[:, None]
    Gm1 = np.zeros((8, 124, C1), np.float32)
    for t in range(8):
        Gm1[t, np.arange(124), t * 4 + np.arange(124) // L1] = 1.0
    Gm2 = np.zeros((2, 128, C1), np.float32)
    for t in range(2):
        Gm2[t, np.arange(128), t * 16 + np.arange(128) // L2] = 1.0

    Wz = np.asarray(W_z, np.float32)
    Wr = np.asarray(W_r, np.float32)
    Wh = np.asarray(W_h, np.float32)

    def stack_zr(W):
        comb = W[0, 0] + W[1, 0] - W[0, 2] - W[1, 2]
        return np.concatenate([comb[:EMB], comb[EMB:], W[0, 1], W[1, 1],
                               2.0 * W[0, 2], 2.0 * W[1, 2]], axis=0)

    W_zr = np.concatenate([stack_zr(Wz), stack_zr(Wr)], axis=1)
    combh = Wh[0, 0] + Wh[1, 0] - Wh[0, 2] - Wh[1, 2]
    W_hs = np.concatenate([
        combh[:EMB], combh[EMB:],
        Wh[0, 1][:EMB], Wh[0, 1][EMB:],
        Wh[1, 1][:EMB], Wh[1, 1][EMB:],
        2.0 * Wh[0, 2][:EMB], 2.0 * Wh[0, 2][EMB:],
        2.0 * Wh[1, 2][:EMB], 2.0 * Wh[1, 2][EMB:],
    ], axis=0)

    consts = {
        "W1t": W1t.astype(ml_dtypes.bfloat16),
        "b1vec": b1vec,
        "W2t": W2t.astype(ml_dtypes.bfloat16),
        "WsumT": WsumT, "b2c": b2c, "Gm1": Gm1, "Gm2": Gm2,
        "gamma1": np.asarray(bn1_gamma, np.float32)[:, None],
        "beta1": np.asarray(bn1_beta, np.float32)[:, None],
        "gamma2": np.asarray(bn2_gamma, np.float32)[:, None],
        "beta2": np.asarray(bn2_beta, np.float32)[:, None],
        "W_zr": W_zr.astype(ml_dtypes.bfloat16),
        "b_zr": np.concatenate([np.asarray(b_z, np.float32),
                                np.asarray(b_r, np.float32)])[:, None],
        "W_hs": W_hs.astype(ml_dtypes.bfloat16),
        "b_hv": np.asarray(b_h, np.float32)[:, None],
    }
    per_core = []
    for c in range(N_CORES):
        m = dict(consts)
        m["xT_sl"] = xT_sl[c]
        m["h_sl"] = h_sl[c]
        m["maskv"] = maskv[c]
        m["gidx"] = gidx_all[c]
        m["loc"] = loc_all[c]
        m["no"] = no_all[c]
        m["ni"] = ni_all[c]
        per_core.append(m)
    return per_core, node_of


# =========================== device program ===============================

def _build_nc(loop=1):
    import contextlib
    from concourse.masks import make_identity

    nc = bacc.Bacc("TRN2", target_bir_lowering=False, debug=False,
                   num_devices=N_CORES, num_swdge_queues=4)
    ap = {}

    def din(name, shape, dt):
        ap[name] = nc.dram_tensor(name, shape, dt, kind="ExternalInput").ap()

    din("xT_sl", [XPAD, NPAD], bf16)
    din("h_sl", [NPAD, EMB], bf16)
    din("maskv", [NPAD], bf16)
    din("gidx", [16, NBLK * CPB * 8], mybir.dt.int16)
    din("loc", [128, NCHUNK], bf16)
    din("no", [128, NCHUNK], bf16)
    din("ni", [128, NCHUNK], bf16)
    din("W1t", [XPAD, F1], bf16)
    din("b1vec", [F1, 1], f32)
    din("W2t", [F1, 2 * 128], bf16)
    din("WsumT", [C1, C1], f32)
    din("b2c", [C1, 1], f32)
    din("Gm1", [8, 124, C1], f32)
    din("Gm2", [2, 128, C1], f32)
    din("gamma1", [C1, 1], f32)
    din("beta1", [C1, 1], f32)
    din("gamma2", [C1, 1], f32)
    din("beta2", [C1, 1], f32)
    din("W_zr", [2560, 512], bf16)
    din("b_zr", [512, 1], f32)
    din("W_hs", [2560, 256], bf16)
    din("b_hv", [256, 1], f32)
    y_ap = nc.dram_tensor("y", [NPAD, EMB], bf16, kind="ExternalOutput").ap()

    xh_mine = [nc.dram_tensor(f"xh_mine{k}", [CROWS, 512], bf16)
               for k in range(NCH)]
    xh_full = nc.dram_tensor("xh_full", [NTOT, 512], bf16, addr_space="Shared")
    t1o_mine = [nc.dram_tensor(f"t1o_mine{k}", [CROWS, 512], bf16)
                for k in range(NCH)]
    t1i_mine = [nc.dram_tensor(f"t1i_mine{k}", [CROWS, 512], bf16)
                for k in range(NCH)]
    t1o_full = nc.dram_tensor("t1o_full", [NTOT, 512], bf16,
                              addr_space="Shared")
    t1i_full = nc.dram_tensor("t1i_full", [NTOT, 512], bf16,
                              addr_space="Shared")
    rh_mine = [nc.dram_tensor(f"rh_mine{k}", [CROWS, EMB], bf16)
               for k in range(NCH)]
    rh_full = nc.dram_tensor("rh_full", [NTOT, EMB], bf16, addr_space="Shared")
    c1o_mine = [nc.dram_tensor(f"c1o_mine{k}", [CROWS, 256], bf16)
                for k in range(NCH)]
    c1i_mine = [nc.dram_tensor(f"c1i_mine{k}", [CROWS, 256], bf16)
                for k in range(NCH)]
    c1o_full = nc.dram_tensor("c1o_full", [NTOT, 256], bf16,
                              addr_space="Shared")
    c1i_full = nc.dram_tensor("c1i_full", [NTOT, 256], bf16,
                              addr_space="Shared")
    ft_zr = nc.dram_tensor("ft_zr", [2560, NPAD], bf16)
    ft_h = nc.dram_tensor("ft_h", [2560, NPAD], bf16)
    bn_part = nc.dram_tensor("bn_part", [C1, 2], f32)
    bn_full = nc.dram_tensor("bn_full", [C1, 2], f32, addr_space="Shared")
    svec_d = nc.dram_tensor("svec_d", [C1], f32)
    ovec_d = nc.dram_tensor("ovec_d", [C1], f32)
    b2p_d = nc.dram_tensor("b2p_d", [C1], f32)

    RG = [list(range(N_CORES))]
    NT = NPAD // 128
    inv1 = 1.0 / (N_NODES * L1)
    inv2 = 1.0 / (N_NODES * L2)

    with tile.TileContext(nc) as tc:
        ctx = contextlib.ExitStack()
        with ctx:
            const_p = ctx.enter_context(tc.tile_pool(name="const", bufs=1))
            work_p = ctx.enter_context(tc.tile_pool(name="work", bufs=2))
            ps_p = ctx.enter_context(tc.tile_pool(name="ps", bufs=2,
                                                  space="PSUM"))
            ps2_p = ctx.enter_context(tc.tile_pool(name="ps2", bufs=2,
                                                   space="PSUM"))
            psg_p = ctx.enter_context(tc.tile_pool(name="psg", bufs=1,
                                                   space="PSUM"))
            stat_p = ctx.enter_context(tc.tile_pool(name="stat", bufs=1))
            ident = const_p.tile([128, 128], f32)
            make_identity(nc, ident[:])
            identb = const_p.tile([128, 128], bf16)
            nc.vector.tensor_copy(identb[:], ident[:])
            iota64_i = const_p.tile([128, 64], i32)
            nc.gpsimd.iota(iota64_i[:], pattern=[[1, 64]], base=0,
                           channel_multiplier=0)
            iota64 = const_p.tile([128, 64], bf16)
            nc.vector.tensor_copy(iota64[:], iota64_i[:])

            prev = None
            for _ in range(loop):
                prev = _emit_body(
                    nc, tc, ap, y_ap, ident, identb, iota64,
                    const_p, work_p, ps_p, ps2_p, psg_p, stat_p,
                    xh_mine, xh_full, t1o_mine, t1i_mine, t1o_full, t1i_full,
                    rh_mine, rh_full,
                    c1o_mine, c1i_mine, c1o_full, c1i_full,
                    ft_zr, ft_h, bn_part, bn_full,
                    svec_d, ovec_d, b2p_d, RG, NT, inv1, inv2, prev)

    nc.compile()
    _split_multi_waits(nc)
    return nc


def _emit_body(nc, tc, ap, y_ap, ident, identb, iota64,
               const_p, work_p, ps_p, ps2_p, psg_p, stat_p,
               xh_mine, xh_full, t1o_mine, t1i_mine, t1o_full, t1i_full,
               rh_mine, rh_full,
               c1o_mine, c1i_mine, c1o_full, c1i_full,
               ft_zr, ft_h, bn_part, bn_full,
               svec_d, ovec_d, b2p_d, RG, NT, inv1, inv2, prev):
    import contextlib

    def pdep(ins_obj, key, reason):
        if prev is not None and key in prev:
            add_dep_helper(ins_obj.ins, prev[key].ins, reason=reason)

    # dma_gather reads indices wrapped in 16 partitions, REPLICATED across
    # the 8 Q7 cores' partition groups (16k:16k+16) -- partition-repeat DMA
    gidx_t = const_p.tile([128, NBLK * CPB * 8], mybir.dt.int16, tag="gidx")
    gidx_src = bass.AP(ap["gidx"].tensor, 0,
                       [[0, 8], [1, 16 * NBLK * CPB * 8]])
    nc.sync.dma_start(gidx_t[:], gidx_src)
    mask_t = const_p.tile([128, NPAD], bf16, tag="mask")
    mask_src = bass.AP(ap["maskv"].tensor, 0, [[0, 128], [1, NPAD]])
    nc.sync.dma_start(mask_t[:], mask_src)

    ctx = contextlib.ExitStack()
    with ctx:
        gate_p = ctx.enter_context(tc.tile_pool(name="gate", bufs=1))
        mid_ctx = contextlib.ExitStack()
        mid_p = mid_ctx.enter_context(tc.tile_pool(name="mid", bufs=1))
        cnn_ctx = contextlib.ExitStack()
        cnn_p = cnn_ctx.enter_context(tc.tile_pool(name="cnn", bufs=1))
        xt_ctx = contextlib.ExitStack()
        xt_p = xt_ctx.enter_context(tc.tile_pool(name="xtp", bufs=1))

        def transpose_to(dst_sb, src_sb, pp, ff):
            """src [pp, ff] -> dst [ff, pp] via PE + copy."""
            pt = ps2_p.tile([128, 128], src_sb.dtype, tag="tr")
            idt = ident if src_sb.dtype == f32 else identb
            nc.tensor.transpose(out=pt[:ff, :pp], in_=src_sb,
                                identity=idt[:pp, :pp])
            nc.vector.tensor_copy(out=dst_sb, in_=pt[:ff, :pp])

        # ---------------- Phase A: CNN ----------------
        xT = xt_p.tile([XPAD, NPAD], bf16, tag="xT")
        nc.sync.dma_start(xT[:], ap["xT_sl"][:])

        W1_t = const_p.tile([XPAD, F1], bf16, tag="W1")
        nc.sync.dma_start(W1_t[:], ap["W1t"][:])
        b1_t = const_p.tile([124, 8, 1], f32, tag="b1")
        nc.sync.dma_start(b1_t[:],
                          ap["b1vec"][:].rearrange("(m p) o -> p m o", p=124))

        a1T = []
        for mc in range(8):
            a1 = cnn_p.tile([124, NPAD], bf16, tag=f"a1_{mc}")
            for nk in range(5):
                ptf = ps_p.tile([128, 512], f32, tag="mm")
                pt = ptf[:124, :]
                nc.tensor.matmul(
                    out=pt[:], lhsT=W1_t[:, mc * 124:(mc + 1) * 124],
                    rhs=xT[:, nk * 512:(nk + 1) * 512],
                    start=True, stop=True)
                nc.scalar.activation(
                    out=a1[:, nk * 512:(nk + 1) * 512], in_=pt[:],
                    func=AF.Relu, bias=b1_t[:, mc, :], scale=1.0)
            nc.vector.tensor_tensor(
                out=a1[:], in0=a1[:],
                in1=mask_t[:124, :], op=OP.mult)
            a1T.append(a1)

        Gm1_t = const_p.tile([124, 8, C1], f32, tag="Gm1")
        nc.sync.dma_start(Gm1_t[:],
                          ap["Gm1"][:].rearrange("t p c -> p t c"))
        bn1_ps = psg_p.tile([C1, 2], f32, tag="zr0", name="bn1ps")
        scratch = cnn_p.tile([124, 512], f32, tag="scr")
        for mc in range(8):
            part = work_p.tile([124, 8], f32, tag="part")
            nc.vector.reduce_sum(part[:, 0:1], a1T[mc][:],
                                 axis=mybir.AxisListType.X)
            for q in range(5):
                nc.scalar.activation(
                    out=scratch[:], in_=a1T[mc][:, q * 512:(q + 1) * 512],
                    func=AF.Square, accum_out=part[:, 3 + q:4 + q])
            nc.vector.reduce_sum(part[:, 1:2], part[:, 3:8],
                                 axis=mybir.AxisListType.X)
            nc.tensor.matmul(out=bn1_ps[:, :], lhsT=Gm1_t[:, mc, :],
                             rhs=part[:, 0:2], start=(mc == 0), stop=(mc == 7))
        bn1_sb = stat_p.tile([C1, 2], f32, tag="bn1")
        nc.vector.tensor_copy(bn1_sb[:], bn1_ps[:])
        dbp1 = nc.gpsimd.dma_start(out=bn_part[:, :], in_=bn1_sb[:])
        pdep(dbp1, "cc_bn2", "bn_part reuse across iters")
        cc_bn1 = nc.gpsimd.collective_compute(
            "AllReduce", OP.add, replica_groups=RG,
            ins=[bn_part[:, :]], outs=[bn_full[:, :]])
        pdep(cc_bn1, "cc3", "collective order across iters")
        pdep(cc_bn1, "ylast", "serialize iterations")

    # (ctx closes nothing yet; pools below)
        def bn_affine(gamma_ap, beta_ap, inv_n, tagp, cc_dep):
            st = stat_p.tile([C1, 2], f32, tag=f"st{tagp}")
            d1 = nc.sync.dma_start(st[:], bn_full[:, :])
            add_dep_helper(d1.ins, cc_dep.ins, reason="after allreduce")
            g_t = stat_p.tile([C1, 1], f32, tag=f"g{tagp}")
            nc.sync.dma_start(g_t[:], gamma_ap[:])
            bta = stat_p.tile([C1, 1], f32, tag=f"b{tagp}")
            nc.sync.dma_start(bta[:], beta_ap[:])
            m = stat_p.tile([C1, 1], f32, tag=f"m{tagp}")
            nc.scalar.activation(out=m[:], in_=st[:, 0:1], func=AF.Copy,
                                 scale=float(inv_n))
            v = stat_p.tile([C1, 1], f32, tag=f"v{tagp}")
            nc.scalar.activation(out=v[:], in_=st[:, 1:2], func=AF.Copy,
                                 scale=float(inv_n))
            msq = stat_p.tile([C1, 1], f32, tag=f"msq{tagp}")
            nc.vector.tensor_tensor(out=msq[:], in0=m[:], in1=m[:],
                                    op=OP.mult)
            nc.vector.tensor_tensor(out=v[:], in0=v[:], in1=msq[:],
                                    op=OP.subtract)
            eps_t = stat_p.tile([C1, 1], f32, tag=f"eps{tagp}")
            nc.gpsimd.memset(eps_t[:], float(BN_EPS))
            ve = stat_p.tile([C1, 1], f32, tag=f"ve{tagp}")
            nc.vector.tensor_tensor(out=ve[:], in0=v[:], in1=eps_t[:],
                                    op=OP.add)
            sd = stat_p.tile([C1, 1], f32, tag=f"sd{tagp}")
            nc.scalar.activation(out=sd[:], in_=ve[:], func=AF.Sqrt)
            rs = stat_p.tile([C1, 1], f32, tag=f"rs{tagp}")
            nc.vector.reciprocal(rs[:], sd[:])
            sv = stat_p.tile([C1, 1], f32, tag=f"sv{tagp}")
            nc.vector.tensor_tensor(out=sv[:], in0=g_t[:], in1=rs[:],
                                    op=OP.mult)
            ov = stat_p.tile([C1, 1], f32, tag=f"ov{tagp}")
            nc.vector.tensor_tensor(out=ov[:], in0=m[:], in1=sv[:],
                                    op=OP.mult)
            nc.vector.tensor_tensor(out=ov[:], in0=bta[:], in1=ov[:],
                                    op=OP.subtract)
            ds = nc.gpsimd.dma_start(out=svec_d[:], in_=sv[:, 0])
            do = nc.gpsimd.dma_start(out=ovec_d[:], in_=ov[:, 0])
            return ds, do, ov

        ds1, do1, ov1 = bn_affine(ap["gamma1"], ap["beta1"], inv1, 1,
                                  cc_bn1)

        W2p = []
        for k in range(8):
            w2k = const_p.tile([124, 2 * 128], bf16, tag=f"w2_{k}")
            nc.sync.dma_start(w2k[:], ap["W2t"][k * 124:(k + 1) * 124, :])
            s1e = work_p.tile([124, 1], f32, tag="s1e")
            src = bass.AP(svec_d.ap().tensor, k * 4, [[1, 4], [0, L1]])
            dr = nc.sync.dma_start(s1e[:], src)
            add_dep_helper(dr.ins, ds1.ins, reason="svec bounce")
            nc.vector.tensor_tensor(out=w2k[:], in0=w2k[:],
                                    in1=s1e[:].to_broadcast([124, 2 * 128]),
                                    op=OP.mult)
            W2p.append(w2k)

        Wsum_t = stat_p.tile([C1, C1], f32, tag="wsum")
        nc.sync.dma_start(Wsum_t[:], ap["WsumT"][:])
        b2ps = psg_p.tile([C1, 1], f32, tag="zr1", name="b2ps")
        nc.tensor.matmul(out=b2ps[:], lhsT=Wsum_t[:], rhs=ov1[:],
                         start=True, stop=True)
        b2p = stat_p.tile([C1, 1], f32, tag="b2p")
        nc.vector.tensor_copy(b2p[:], b2ps[:])
        b2c_t = stat_p.tile([C1, 1], f32, tag="b2c")
        nc.sync.dma_start(b2c_t[:], ap["b2c"][:])
        nc.vector.tensor_tensor(out=b2p[:], in0=b2p[:], in1=b2c_t[:],
                                op=OP.add)
        db2 = nc.gpsimd.dma_start(out=b2p_d[:], in_=b2p[:, 0])
        b2e = []
        for t in range(2):
            b2et = stat_p.tile([128, 1], f32, tag=f"b2e{t}")
            src = bass.AP(b2p_d.ap().tensor, t * 16, [[1, 16], [0, L2]])
            dr = nc.sync.dma_start(b2et[:], src)
            add_dep_helper(dr.ins, db2.ins, reason="b2p bounce")
            b2e.append(b2et)

        a2T = []
        for mt in range(2):
            a2 = mid_p.tile([128, NPAD], bf16, tag=f"a2_{mt}")
            for nk in range(5):
                pt = ps_p.tile([128, 512], f32, tag="mm")
                for k in range(8):
                    nc.tensor.matmul(
                        out=pt[:],
                        lhsT=W2p[k][:, mt * 128:(mt + 1) * 128],
                        rhs=a1T[k][:, nk * 512:(nk + 1) * 512],
                        start=(k == 0), stop=(k == 7))
                nc.scalar.activation(
                    out=a2[:, nk * 512:(nk + 1) * 512], in_=pt[:],
                    func=AF.Relu, bias=b2e[mt][:], scale=1.0)
            nc.vector.tensor_tensor(
                out=a2[:], in0=a2[:],
                in1=mask_t[:], op=OP.mult)
            a2T.append(a2)

        Gm2_t = const_p.tile([128, 2, C1], f32, tag="Gm2")
        nc.sync.dma_start(Gm2_t[:],
                          ap["Gm2"][:].rearrange("t p c -> p t c"))
        bn2_ps = psg_p.tile([C1, 2], f32, tag="zr2", name="bn2ps")
        scratch2 = mid_p.tile([128, 512], f32, tag="scr2")
        for mt in range(2):
            part = work_p.tile([128, 8], f32, tag="part2")
            nc.vector.reduce_sum(part[:, 0:1], a2T[mt][:],
                                 axis=mybir.AxisListType.X)
            for q in range(5):
                nc.scalar.activation(
                    out=scratch2[:], in_=a2T[mt][:, q * 512:(q + 1) * 512],
                    func=AF.Square, accum_out=part[:, 3 + q:4 + q])
            nc.vector.reduce_sum(part[:, 1:2], part[:, 3:8],
                                 axis=mybir.AxisListType.X)
            nc.tensor.matmul(out=bn2_ps[:, :], lhsT=Gm2_t[:, mt, :],
                             rhs=part[:, 0:2], start=(mt == 0), stop=(mt == 1))
        bn2_sb = stat_p.tile([C1, 2], f32, tag="bn2")
        nc.vector.tensor_copy(bn2_sb[:], bn2_ps[:])
        dbp2 = nc.gpsimd.dma_start(out=bn_part[:, :], in_=bn2_sb[:])
        add_dep_helper(dbp2.ins, cc_bn1.ins, reason="bn_part reuse")
        cc_bn2 = nc.gpsimd.collective_compute(
            "AllReduce", OP.add, replica_groups=RG,
            ins=[bn_part[:, :]], outs=[bn_full[:, :]])
        add_dep_helper(cc_bn2.ins, cc_bn1.ins, reason="collective order")

        ds2, do2, _ = bn_affine(ap["gamma2"], ap["beta2"], inv2, 2, cc_bn2)

        xhT = []
        for mt in range(2):
            s2et = stat_p.tile([128, 1], f32, tag=f"s2e{mt}")
            dr1 = nc.sync.dma_start(
                s2et[:], bass.AP(svec_d.ap().tensor, mt * 16,
                                 [[1, 16], [0, L2]]))
            add_dep_helper(dr1.ins, ds2.ins, reason="svec2 bounce")
            o2et = stat_p.tile([128, 1], f32, tag=f"o2e{mt}")
            dr2 = nc.sync.dma_start(
                o2et[:], bass.AP(ovec_d.ap().tensor, mt * 16,
                                 [[1, 16], [0, L2]]))
            add_dep_helper(dr2.ins, do2.ins, reason="ovec2 bounce")
            xt = a2T[mt]
            nc.vector.tensor_tensor(
                out=xt[:], in0=xt[:],
                in1=s2et[:].to_broadcast([128, NPAD]), op=OP.mult)
            nc.vector.tensor_tensor(
                out=xt[:], in0=xt[:],
                in1=o2et[:].to_broadcast([128, NPAD]), op=OP.add)
            nc.vector.tensor_tensor(
                out=xt[:], in0=xt[:],
                in1=mask_t[:], op=OP.mult)
            xhT.append(xt)
            nc.sync.dma_start(ft_zr[mt * 128:(mt + 1) * 128, :], xt[:])
            nc.sync.dma_start(ft_h[mt * 128:(mt + 1) * 128, :], xt[:])

        xt_ctx.close()
        cnn_ctx.close()
        h0T = []
        for mt in range(2):
            h0T_t = gate_p.tile([128, NPAD], bf16, tag=f"h0T_{mt}")
            h0T.append(h0T_t)

        TPC = NT // NCH        # t-tiles per collective chunk
        cc_prev = cc_bn2
        cc0s = []
        for t in range(NT):
            ck, r0 = t // TPC, (t % TPC) * 128
            hb = work_p.tile([128, EMB], bf16, tag="h0b")
            nc.sync.dma_start(hb[:], ap["h_sl"][t * 128:(t + 1) * 128, :])
            dw = nc.sync.dma_start(xh_mine[ck][r0:r0 + 128, 256:512], hb[:])
            pdep(dw, "cc0", "xh_mine reuse across iters")
            for mt in range(2):
                transpose_to(h0T[mt][:, t * 128:(t + 1) * 128],
                             hb[:, mt * 128:(mt + 1) * 128], 128, 128)
                trd = work_p.tile([128, 128], bf16, tag="trd")
                transpose_to(trd[:], xhT[mt][:, t * 128:(t + 1) * 128],
                             128, 128)
                dw = nc.sync.dma_start(
                    xh_mine[ck][r0:r0 + 128,
                                mt * 128:(mt + 1) * 128], trd[:])
                pdep(dw, "cc0", "xh_mine reuse across iters")
            if t % TPC == TPC - 1:
                cc = nc.gpsimd.collective_compute(
                    "AllGather", OP.bypass, replica_groups=RG,
                    ins=[xh_mine[ck][:, :]],
                    outs=[xh_full[ck * GROWS:(ck + 1) * GROWS, :]])
                add_dep_helper(cc.ins, cc_prev.ins, reason="collective order")
                cc_prev = cc
                cc0s.append(cc)
        for mt in range(2):
            nc.sync.dma_start(ft_zr[256 + mt * 128:256 + (mt + 1) * 128, :],
                              h0T[mt][:])
        mid_ctx.close()
        sres_p = ctx.enter_context(tc.tile_pool(name="sres", bufs=1))
        gath_p = ctx.enter_context(tc.tile_pool(name="gath", bufs=3))
        cc0 = cc0s[-1]

        # ---- on-device S build from compact per-edge loc/no/ni ----
        loc_t = work_p.tile([128, NCHUNK], bf16, tag="loc")
        nc.sync.dma_start(loc_t[:], ap["loc"][:])
        no_t = work_p.tile([128, NCHUNK], bf16, tag="no")
        nc.sync.dma_start(no_t[:], ap["no"][:])
        ni_t = work_p.tile([128, NCHUNK], bf16, tag="ni")
        nc.sync.dma_start(ni_t[:], ap["ni"][:])
        S_res = sres_p.tile([128, NCHUNK * 128], bf16, tag="S")
        Sv = S_res[:].rearrange("p (ch c) -> p ch c", c=128)
        iota_b = iota64[:].unsqueeze(1).to_broadcast([128, NCHUNK, 64])
        loc_b = loc_t[:].unsqueeze(2).to_broadcast([128, NCHUNK, 64])
        no_b = no_t[:].unsqueeze(2).to_broadcast([128, NCHUNK, 64])
        ni_b = ni_t[:].unsqueeze(2).to_broadcast([128, NCHUNK, 64])
        nc.vector.tensor_tensor(out=Sv[:, :, 64:128], in0=iota_b, in1=loc_b,
                                op=OP.is_equal)
        nc.vector.tensor_tensor(out=Sv[:, :, 0:64], in0=Sv[:, :, 64:128],
                                in1=no_b, op=OP.mult)
        nc.vector.tensor_tensor(out=Sv[:, :, 64:128], in0=Sv[:, :, 64:128],
                                in1=ni_b, op=OP.mult)

        # ------------- propagation rounds -------------
        def prop_round(src_dram, src_w, dual, out_mine, oT, iT, extra,
                       dep_cc, tagr, cc_emit=None):
            BPC = NBLK // NCH   # blocks per collective chunk
            W = src_w if dual else src_w // 2
            for b in range(NBLK):
                if dual:
                    ptf = ps_p.tile([128, 512], f32, tag="mm")
                    pt = ptf[:, :W]
                else:
                    # two psum tiles: one accumulation group per bank
                    pto_t = ps_p.tile([128, 512], f32, tag="mm", name="pto")
                    pti_t = ps_p.tile([128, 512], f32, tag="mm", name="pti")
                    pto = pto_t[0:64, :W]
                    pti = pti_t[0:64, :W]
                if dual:
                    gfull = gath_p.tile([128, CPB * src_w], bf16, tag="g",
                                        name="gt")
                    g3 = gfull[:].rearrange("p (a b) -> p a b", b=src_w)
                    gi = nc.gpsimd.dma_gather(
                        g3, src_dram[:, :], gidx_t[:, b * 64:(b + 1) * 64],
                        CPB * 128, CPB * 128, src_w,
                        queue_num=b % 4)
                    if dep_cc is not None:
                        add_dep_helper(gi.ins, dep_cc.ins,
                                       reason="gather after allgather")
                    for j in range(CPB):
                        ch = b * CPB + j
                        nc.tensor.matmul(
                            out=pt[:],
                            lhsT=S_res[:, ch * 128:(ch + 1) * 128],
                            rhs=g3[:, j, :], start=(j == 0),
                            stop=(j == CPB - 1))
                else:
                    # separate o/i source matrices with separate collective
                    # deps: out-dir matmuls overlap the in-dir AllGather
                    src_o, src_i = src_dram
                    dep_o, dep_i = dep_cc
                    g3d = []
                    for (srcM, dep, q) in ((src_o, dep_o, 0),
                                           (src_i, dep_i, 1)):
                        gfull = gath_p.tile([128, CPB * W], bf16, tag="g",
                                            name="gt")
                        g3 = gfull[:].rearrange("p (a b) -> p a b", b=W)
                        gi = nc.gpsimd.dma_gather(
                            g3, srcM[:, :], gidx_t[:, b * 64:(b + 1) * 64],
                            CPB * 128, CPB * 128, W,
                            queue_num=(b + q) % 4)
                        add_dep_helper(gi.ins, dep.ins,
                                       reason="gather after allgather")
                        g3d.append(g3)
                    for j in range(CPB):
                        ch = b * CPB + j
                        nc.tensor.matmul(
                            out=pto[:],
                            lhsT=S_res[:, ch * 128:ch * 128 + 64],
                            rhs=g3d[0][:, j, :], start=(j == 0),
                            stop=(j == CPB - 1))
                    for j in range(CPB):
                        ch = b * CPB + j
                        nc.tensor.matmul(
                            out=pti[:],
                            lhsT=S_res[:, ch * 128 + 64:(ch + 1) * 128],
                            rhs=g3d[1][:, j, :], start=(j == 0),
                            stop=(j == CPB - 1))
                (dr_o, base_o) = oT
                (dr_i, base_i) = iT
                ck, rb = b // BPC, (b % BPC) * 64
                if dual:
                    blk_full = work_p.tile([128, 512], bf16, tag="bs")
                    blk_sb = blk_full[:, :W]
                    nc.vector.tensor_copy(blk_sb[:], pt[:])
                    if out_mine is not None:
                        out_o, out_i = out_mine
                        nc.sync.dma_start(
                            out_o[ck][rb:rb + 64, 0:W],
                            blk_sb[0:64, :])
                        nc.sync.dma_start(
                            out_i[ck][rb:rb + 64, 0:W],
                            blk_sb[64:128, :])
                    for f in range(W // 128):
                        trd = work_p.tile([128, 128], bf16, tag="trd")
                        transpose_to(trd[:], blk_sb[:, f * 128:(f + 1) * 128],
                                     128, 128)
                        nc.sync.dma_start(
                            dr_o[base_o + f * 128:base_o + (f + 1) * 128,
                                 b * 64:(b + 1) * 64], trd[:, 0:64])
                        nc.sync.dma_start(
                            dr_i[base_i + f * 128:base_i + (f + 1) * 128,
                                 b * 64:(b + 1) * 64], trd[:, 64:128])
                        if extra is not None and f < 2:
                            (er_o, ebase_o), (er_i, ebase_i) = extra
                            nc.sync.dma_start(
                                er_o[ebase_o + f * 128:ebase_o + (f + 1) * 128,
                                     b * 64:(b + 1) * 64], trd[:, 0:64])
                            nc.sync.dma_start(
                                er_i[ebase_i + f * 128:ebase_i + (f + 1) * 128,
                                     b * 64:(b + 1) * 64], trd[:, 64:128])
                else:
                    blk_o_t = work_p.tile([128, 512], bf16, tag="bs",
                                          name="blko")
                    blk_i_t = work_p.tile([128, 512], bf16, tag="bsi",
                                          name="blki")
                    blk_o = blk_o_t[0:64, :W]
                    blk_i = blk_i_t[0:64, :W]
                    nc.vector.tensor_copy(blk_o[:], pto[:])
                    nc.vector.tensor_copy(blk_i[:], pti[:])
                    for f in range(W // 128):
                        tro = work_p.tile([128, 64], bf16, tag="trdo")
                        transpose_to(tro[:], blk_o[:, f * 128:(f + 1) * 128],
                                     64, 128)
                        tri = work_p.tile([128, 64], bf16, tag="trdi")
                        transpose_to(tri[:], blk_i[:, f * 128:(f + 1) * 128],
                                     64, 128)
                        nc.sync.dma_start(
                            dr_o[base_o + f * 128:base_o + (f + 1) * 128,
                                 b * 64:(b + 1) * 64], tro[:])
                        nc.sync.dma_start(
                            dr_i[base_i + f * 128:base_i + (f + 1) * 128,
                                 b * 64:(b + 1) * 64], tri[:])
                        if extra is not None and f < 2:
                            (er_o, ebase_o), (er_i, ebase_i) = extra
                            nc.sync.dma_start(
                                er_o[ebase_o + f * 128:ebase_o + (f + 1) * 128,
                                     b * 64:(b + 1) * 64], tro[:])
                            nc.sync.dma_start(
                                er_i[ebase_i + f * 128:ebase_i + (f + 1) * 128,
                                     b * 64:(b + 1) * 64], tri[:])
                if cc_emit is not None and (b + 1) % BPC == 0:
                    cc_emit(ck)

        # R1: T1o^T -> ft_zr 512:1024 ; T1i^T -> ft_zr 1024:1536
        #     A1o^T -> ft_h 512:768  ; A1i^T -> ft_h 1024:1280
        def make_cc_emit(mine_o, full_o, mine_i, full_i, ccs):
            def emit(ck):
                nonlocal cc_prev
                for mine, full in ((mine_o, full_o), (mine_i, full_i)):
                    cc = nc.gpsimd.collective_compute(
                        "AllGather", OP.bypass, replica_groups=RG,
                        ins=[mine[ck][:, :]],
                        outs=[full[ck * GROWS:(ck + 1) * GROWS, :]])
                    add_dep_helper(cc.ins, cc_prev.ins,
                                   reason="collective order")
                    cc_prev = cc
                    ccs.append(cc)
            return emit

        cc1s = []
        prop_round(xh_full, 512, True, (t1o_mine, t1i_mine),
                   (ft_zr, 512), (ft_zr, 1024),
                   ((ft_h, 512), (ft_h, 1024)), cc0, "r1",
                   cc_emit=make_cc_emit(t1o_mine, t1o_full,
                                        t1i_mine, t1i_full, cc1s))
        cc1 = cc1s[-1]

        # R2: T2o^T -> ft_zr 1536:2048 ; T2i^T -> ft_zr 2048:2560
        #     A2o^T -> ft_h 1536:1792 ; A2i^T -> ft_h 2048:2304
        prop_round((t1o_full, t1i_full), 1024, False, None,
                   (ft_zr, 1536), (ft_zr, 2048),
                   ((ft_h, 1536), (ft_h, 2048)), (cc1s[0], cc1s[1]), "r2")

        # ------------- dense ZR gate -------------
        zrw_ctx = contextlib.ExitStack()
        zrw_p = zrw_ctx.enter_context(tc.tile_pool(name="zrw", bufs=1))
        Wzr_t = zrw_p.tile([128, 20, 512], bf16, tag="wzr")
        nc.sync.dma_start(
            Wzr_t[:], ap["W_zr"][:].rearrange("(k p) o -> p k o", p=128))
        bzr_t = const_p.tile([128, 4, 1], f32, tag="bzr")
        nc.sync.dma_start(
            bzr_t[:], ap["b_zr"][:].rearrange("(m p) o -> p m o", p=128))
        zT = []
        rT = []
        for i in range(2):
            zT_t = gate_p.tile([128, NPAD], bf16, tag=f"zT{i}")
            zT.append(zT_t)
            rT_t = gate_p.tile([128, NPAD], bf16, tag=f"rT{i}")
            rT.append(rT_t)
        for nk in range(5):
            pts = [psg_p.tile([128, 512], f32, tag=f"zr{ok}",
                              name=f"pzr{ok}") for ok in range(4)]
            for k in range(20):
                rhs = work_p.tile([128, 512], bf16, tag="fzr")
                nc.sync.dma_start(
                    rhs[:], ft_zr[k * 128:(k + 1) * 128,
                                  nk * 512:(nk + 1) * 512])
                for ok in range(4):
                    nc.tensor.matmul(
                        out=pts[ok][:],
                        lhsT=Wzr_t[:, k, ok * 128:(ok + 1) * 128],
                        rhs=rhs[:], start=(k == 0), stop=(k == 19))
            for ok in range(4):
                dst = zT[ok] if ok < 2 else rT[ok - 2]
                nc.scalar.activation(
                    out=dst[:, nk * 512:(nk + 1) * 512], in_=pts[ok][:],
                    func=AF.Sigmoid, bias=bzr_t[:, ok, :], scale=1.0)

        rhs_t = []
        for mt in range(2):
            rh = gate_p.tile([128, NPAD], bf16, tag=f"rh{mt}",
                             name=f"rh{mt}")
            nc.vector.tensor_tensor(out=rh[:], in0=rT[mt][:],
                                    in1=h0T[mt][:], op=OP.mult)
            nc.sync.dma_start(
                ft_h[256 + mt * 128:256 + (mt + 1) * 128, :], rh[:])
            rhs_t.append(rh)

        cc2s = []
        for t in range(NT):
            ck, r0 = t // TPC, (t % TPC) * 128
            for mt in range(2):
                trd = work_p.tile([128, 128], bf16, tag="trrh")
                transpose_to(trd[:], rhs_t[mt][:, t * 128:(t + 1) * 128],
                             128, 128)
                nc.sync.dma_start(
                    rh_mine[ck][r0:r0 + 128,
                                mt * 128:(mt + 1) * 128], trd[:])
            if t % TPC == TPC - 1:
                cc = nc.gpsimd.collective_compute(
                    "AllGather", OP.bypass, replica_groups=RG,
                    ins=[rh_mine[ck][:, :]],
                    outs=[rh_full[ck * GROWS:(ck + 1) * GROWS, :]])
                add_dep_helper(cc.ins, cc_prev.ins, reason="collective order")
                cc_prev = cc
                cc2s.append(cc)

        zrw_ctx.close()
        cc2 = cc2s[-1]

        # R3a: C1o^T -> ft_h 768:1024 ; C1i^T -> ft_h 1280:1536
        cc3s = []
        prop_round(rh_full, 256, True, (c1o_mine, c1i_mine),
                   (ft_h, 768), (ft_h, 1280), None, cc2, "r3a",
                   cc_emit=make_cc_emit(c1o_mine, c1o_full,
                                        c1i_mine, c1i_full, cc3s))
        cc3 = cc3s[-1]

        # R3b: C2o^T -> ft_h 1792:2048 ; C2i^T -> ft_h 2304:2560
        prop_round((c1o_full, c1i_full), 512, False, None,
                   (ft_h, 1792), (ft_h, 2304), None, (cc3s[0], cc3s[1]),
                   "r3b")

        # ------------- dense H gate + GRU output -------------
        fin_ctx = contextlib.ExitStack()
        fin_p = fin_ctx.enter_context(tc.tile_pool(name="fin", bufs=1))
        Whs_t = fin_p.tile([128, 20, 256], bf16, tag="whs")
        nc.sync.dma_start(
            Whs_t[:], ap["W_hs"][:].rearrange("(k p) o -> p k o", p=128))
        bh_t = const_p.tile([128, 2, 1], f32, tag="bh")
        nc.sync.dma_start(
            bh_t[:], ap["b_hv"][:].rearrange("(m p) o -> p m o", p=128))
        ylast = None
        for nk in range(5):
            pts = [psg_p.tile([128, 512], f32, tag=f"zr{ok}",
                              name=f"ph{ok}") for ok in range(2)]
            for k in range(20):
                rhs = work_p.tile([128, 512], bf16, tag="fh")
                nc.sync.dma_start(
                    rhs[:], ft_h[k * 128:(k + 1) * 128,
                                 nk * 512:(nk + 1) * 512])
                for ok in range(2):
                    nc.tensor.matmul(
                        out=pts[ok][:],
                        lhsT=Whs_t[:, k, ok * 128:(ok + 1) * 128],
                        rhs=rhs[:], start=(k == 0), stop=(k == 19))
            for ok in range(2):
                sl = slice(nk * 512, (nk + 1) * 512)
                hT = work_p.tile([128, 512], f32, tag="hTn")
                nc.scalar.activation(
                    out=hT[:], in_=pts[ok][:],
                    func=AF.Tanh, bias=bh_t[:, ok, :], scale=1.0)
                diff = work_p.tile([128, 512], f32, tag="diffn")
                nc.vector.tensor_tensor(out=diff[:], in0=h0T[ok][:, sl],
                                        in1=hT[:], op=OP.subtract)
                nc.vector.tensor_tensor(out=diff[:], in0=zT[ok][:, sl],
                                        in1=diff[:], op=OP.mult)
                nc.vector.tensor_tensor(out=hT[:], in0=hT[:], in1=diff[:],
                                        op=OP.add)
                nc.scalar.activation(out=hT[:], in_=hT[:], func=AF.Relu)
                for tt in range(4):
                    t = nk * 4 + tt
                    pt2 = ps2_p.tile([128, 128], f32, tag="tr")
                    nc.tensor.transpose(
                        out=pt2[:], in_=hT[:, tt * 128:(tt + 1) * 128],
                        identity=ident[:])
                    of = work_p.tile([128, 128], bf16, tag="yf")
                    nc.vector.tensor_copy(of[:], pt2[:])
                    ylast = nc.sync.dma_start(
                        y_ap[t * 128:(t + 1) * 128,
                             ok * 128:(ok + 1) * 128], of[:])
        fin_ctx.close()

    return {"cc_bn2": cc_bn2, "cc0": cc0, "cc3": cc3, "ylast": ylast}


_CACHE = {}


def _get_nc(loop=1):
    key = f"nc{loop}"
    if key not in _CACHE:
        _CACHE[key] = _build_nc(loop)
    return _CACHE[key]


def kernel(**inputs) -> np.ndarray:
    per_core, node_of = _host_prep(**inputs)
    nc = _get_nc()
    res = bass_utils.run_bass_kernel_spmd(
        nc, per_core, core_ids=list(range(N_CORES)))
    out = np.zeros((N_NODES, EMB), np.float32)
    for c in range(N_CORES):
        m = node_of[c] >= 0
        out[node_of[c][m]] = res.results[c]["y"][m].astype(np.float32)
    return out
